# revision 14
# baseline (speedup 1.0000x reference)
"""Trainium2 Bass kernel for AudioResNetPCEN.

Data-parallel over 8 NeuronCores: 4 samples/core. Per core:
  frames (host im2col, bf16) -> DFT via matmul (re/im) -> |.| -> mel matmul
  -> PCEN (IIR via DVE tensor_tensor_scan, exact recurrence) -> input BN
  -> conv1 7x7/s2 via host-built banded matrices (contraction over mel axis)
  -> maxpool 3x3/s2 (shifted-view DVE max)
  -> ResNet34 basic blocks: 3x3 convs as 9 shifted 1x1 matmuls, BN folded
     into weights on host, bias+ReLU fused into the PSUM->SBUF activation
  -> avgpool (tensor_reduce) -> fc matmul -> [10, 4] out per core.
"""
import os
import sys
import numpy as np

sys.path.insert(0, "/opt/trn_rl_repo")

import ml_dtypes
from concourse import bass, bacc, tile, mybir
from concourse.bass_utils import run_bass_kernel_spmd

BF16 = ml_dtypes.bfloat16

SR = 32000; NFFT = 1024; HOP = 320; NMELS = 64
EPS = 1e-6; BN_EPS = 1e-5
S = HOP / (SR * 0.06)
T = 401          # frames per sample
B = 4            # samples per core
NCORES = 8
NF = 512         # freq bins used (bin 512 has zero mel weight)

F32 = mybir.dt.float32
BF = mybir.dt.bfloat16
AF = mybir.ActivationFunctionType
ALU = mybir.AluOpType
AX = mybir.AxisListType

# ResNet34 config: (cout, nblocks) per layer; spatial geometry per layer input
CFG = [(64, 3), (128, 4), (256, 6), (512, 3)]
# geometry AFTER each layer: layer i blocks operate at OUT spatial size
GEOM = {1: (16, 101), 2: (8, 51), 3: (4, 26), 4: (2, 13)}


# ---------------------------------------------------------------- host prep
def _hz_to_mel(f):
    return 2595.0 * np.log10(1.0 + f / 700.0)


def _mel_to_hz(m):
    return 700.0 * (10.0 ** (m / 2595.0) - 1.0)


def mel_fbank_np():
    n_freqs = NFFT // 2 + 1
    all_freqs = np.linspace(0.0, SR // 2, n_freqs)
    m_pts = np.linspace(_hz_to_mel(20.0), _hz_to_mel(16000.0), NMELS + 2)
    f_pts = _mel_to_hz(m_pts)
    f_diff = f_pts[1:] - f_pts[:-1]
    slopes = f_pts[None, :] - all_freqs[:, None]
    down = -slopes[:, :-2] / f_diff[:-1]
    up = slopes[:, 2:] / f_diff[1:]
    return np.maximum(0.0, np.minimum(down, up)).astype(np.float32)  # [513, 64]


def build_dft():
    n = np.arange(NFFT)
    win = 0.5 * (1.0 - np.cos(2.0 * np.pi * n / NFFT))
    k = np.arange(NF)
    ang = 2.0 * np.pi * np.outer(n, k) / NFFT
    Cre = (win[:, None] * np.cos(ang)).astype(np.float32)
    Cim = (win[:, None] * np.sin(ang)).astype(np.float32)
    # dftm[p, c, ri, k] = C[128c+p, k]
    dftm = np.stack([Cre.reshape(8, 128, NF), Cim.reshape(8, 128, NF)], axis=2)
    return np.ascontiguousarray(dftm.transpose(1, 0, 2, 3)).astype(BF16)  # [128,8,2,512]


def conv1_mats(w1f):
    """w1f [64,1,7,7] folded -> paired [4, 128, 2048]: pair j rows 0:64 = B_{2j},
    rows 64:128 = B_{2j+1} (zero for j=3); B_kw[ih, oc*32+oh] = w1f[oc,0,kh,kw]."""
    bm = np.zeros((7, 64, 2048), np.float32)
    kh = np.arange(7)
    for kw in range(7):
        for oc in range(64):
            for oh in range(32):
                ih = 2 * oh + kh - 3
                m = (ih >= 0) & (ih < 64)
                bm[kw, ih[m], oc * 32 + oh] = w1f[oc, 0, kh[m], kw]
    out = np.zeros((4, 128, 2048), np.float32)
    for j in range(4):
        out[j, :64] = bm[2 * j]
        if 2 * j + 1 < 7:
            out[j, 64:] = bm[2 * j + 1]
    return out.astype(BF16)


def fold(w, g):
    return (np.asarray(w, np.float32)
            * (np.asarray(g, np.float32) / np.sqrt(1.0 + BN_EPS))[:, None, None, None])


def conv_names():
    names = []
    cin = 64
    for li, (c, n) in enumerate(CFG):
        for bi in range(n):
            stride = 2 if (li > 0 and bi == 0) else 1
            names.append((f"w_{li}_{bi}_1", cin, c, 3, stride))
            names.append((f"w_{li}_{bi}_2", c, c, 3, 1))
            if stride != 1 or cin != c:
                names.append((f"w_{li}_{bi}_d", cin, c, 1, stride))
            cin = c
    return names


def pack_conv(wf):
    """wf [Cout, Cin, k, k] -> [CT, 128 (or cin), KK, Cout] array"""
    co, ci, k, _ = wf.shape
    P = min(ci, 128)
    CT = (ci + 127) // 128
    out = np.zeros((CT, P, k * k, co), np.float32)
    for ct in range(CT):
        sl = wf[:, ct * 128:(ct + 1) * 128]  # [co, P, k, k]
        out[ct] = sl.reshape(co, P, k * k).transpose(1, 2, 0)
    return out.astype(BF16)


def pack_conv_paired(wf):
    """wf [Cout, 64, 3, 3] -> [1, 128, 6, Cout]; slot s=kh*2+j:
    j=0: rows 0:64 tap kw=0, rows 64:128 tap kw=1; j=1: rows 0:64 tap kw=2."""
    co = wf.shape[0]
    assert wf.shape[1] == 64 and wf.shape[2] == 3
    out = np.zeros((1, 128, 6, co), np.float32)
    for kh in range(3):
        out[0, :64, kh * 2 + 0] = wf[:, :, kh, 0].T
        out[0, 64:, kh * 2 + 0] = wf[:, :, kh, 1].T
        out[0, :64, kh * 2 + 1] = wf[:, :, kh, 2].T
    return out.astype(BF16)


PAIRED = set()
for _bi in range(3):
    PAIRED.add(f"w_0_{_bi}_1"); PAIRED.add(f"w_0_{_bi}_2")
PAIRED.add("w_1_0_1")


def host_prep(params):
    """-> (const_map {name: np.ndarray}, meta)"""
    p = params
    cmap = {}
    cmap["dftm"] = build_dft()
    fb = mel_fbank_np()
    assert abs(fb[512]).max() == 0.0
    fbank = np.zeros((128, 4, 64), np.float32)
    for ct in range(4):
        fbank[:, ct] = fb[ct * 128:(ct + 1) * 128]
    cmap["fbank"] = fbank.astype(BF16)

    w1f = fold(p["conv1"], p["g1"])
    cmap["w1m"] = conv1_mats(w1f)

    # biases: one fp32 [128, NB] array; column map
    cols = {}
    bias_cols = []
    b1 = np.asarray(p["b1"], np.float32)
    for mt in range(16):
        col = np.zeros(128, np.float32)
        for pp in range(128):
            col[pp] = b1[4 * mt + pp // 32]
        bias_cols.append(col)
    cols["conv1"] = 0

    def add_bias(name, bias):
        cols[name] = len(bias_cols)
        co = bias.shape[0]
        for cot in range((co + 127) // 128):
            col = np.zeros(128, np.float32)
            seg = bias[cot * 128:(cot + 1) * 128]
            col[:len(seg)] = seg
            bias_cols.append(col)

    cin = 64
    for li, (c, n) in enumerate(CFG):
        for bi in range(n):
            stride = 2 if (li > 0 and bi == 0) else 1
            bp = p["layers"][li][bi]
            pk1 = pack_conv_paired if f"w_{li}_{bi}_1" in PAIRED else pack_conv
            pk2 = pack_conv_paired if f"w_{li}_{bi}_2" in PAIRED else pack_conv
            cmap[f"w_{li}_{bi}_1"] = pk1(fold(bp["w1"], bp["g1"]))
            add_bias(f"w_{li}_{bi}_1", np.asarray(bp["b1"], np.float32))
            cmap[f"w_{li}_{bi}_2"] = pk2(fold(bp["w2"], bp["g2"]))
            add_bias(f"w_{li}_{bi}_2", np.asarray(bp["b2"], np.float32))
            if stride != 1 or cin != c:
                cmap[f"w_{li}_{bi}_d"] = pack_conv(fold(bp["wd"], bp["gd"]))
                add_bias(f"w_{li}_{bi}_d", np.asarray(bp["bd"], np.float32))
            cin = c
    # fc
    fcb = np.zeros(128, np.float32)
    fcb[:10] = np.asarray(p["fc_b"], np.float32)
    cols["fc"] = len(bias_cols)
    bias_cols.append(fcb)
    cmap["biases"] = np.stack(bias_cols, axis=1).astype(np.float32)  # [128, NB]

    fcw = np.zeros((128, 4, 10), np.float32)
    w = np.asarray(p["fc_w"], np.float32) / 26.0  # fold avgpool 1/(2*13)
    for ct in range(4):
        fcw[:, ct] = w[:, ct * 128:(ct + 1) * 128].T
    cmap["fcw"] = fcw

    # PCEN / input-bn scalars
    alpha = float(np.clip(np.asarray(p["alpha"]), 0.01, 0.99))
    delta = float(abs(np.asarray(p["delta"])) + EPS)
    r = float(np.clip(np.asarray(p["r"]), 0.01, 1.0))
    assert abs(r - 0.5) < 1e-6, "kernel assumes r=0.5 (sqrt)"
    s0 = float(np.asarray(p["inbn_g"])[0] / np.sqrt(1.0 + BN_EPS))
    c0 = float(np.asarray(p["inbn_b"])[0] - (delta ** r) * s0)
    meta = dict(alpha=alpha, delta=delta, s0=s0, c0=c0, cols=cols)
    return cmap, meta


def frames_for_shard(xs):
    """xs [B, 128000] fp32 -> [128, 8, B*T] bf16 (frames_T[p,c,(b,t)] = xp[b, 320t+128c+p])"""
    xp = np.pad(xs, ((0, 0), (512, 512)), mode="reflect")
    sw = np.lib.stride_tricks.sliding_window_view(xp, NFFT, axis=1)  # [B, L-1023, 1024]
    fr = sw[:, ::HOP][:, :T]                    # [B, T, 1024]
    fr = fr.transpose(2, 0, 1).reshape(8, 128, B * T)  # [c, p, bt]
    return np.ascontiguousarray(fr.transpose(1, 0, 2)).astype(BF16)


# ---------------------------------------------------------------- graph
def build_graph(cmap, meta, dbg=0):
    nc = bacc.Bacc("TRN2", target_bir_lowering=False, debug=False,
                   num_devices=NCORES)
    cols = meta["cols"]

    for v in (float(EPS), float(meta["delta"])):
        if (F32, v) not in nc.const_aps.aps:
            t_ = nc.alloc_sbuf_tensor(f"const-f32-{v}", [128, 1], F32)
            nc.gpsimd.memset(t_.ap(), v)
            nc.const_aps.aps[(F32, v)] = t_.ap()

    ext = {}
    ext["frames"] = nc.dram_tensor("frames", [128, 8, B * T], BF, kind="ExternalInput")
    for name, arr in cmap.items():
        dt = BF if arr.dtype == BF16 else F32
        ext[name] = nc.dram_tensor(name, list(arr.shape), dt, kind="ExternalInput")
    out_ext = nc.dram_tensor("out", [10, B], F32, kind="ExternalOutput")
    if dbg == 1:
        dbg_ext = nc.dram_tensor("dbg", [64, B, 407], BF, kind="ExternalOutput")
    elif dbg == 2:
        dbg_ext = nc.dram_tensor("dbg", [64, 1, B, 18, 103], BF, kind="ExternalOutput")
    elif dbg == 3:
        dbg_ext = nc.dram_tensor("dbg", [64, 1, B, 18, 103], BF, kind="ExternalOutput")

    with tile.TileContext(nc) as tc:
        import contextlib
        stack = contextlib.ExitStack()
        with stack:
            cpool = stack.enter_context(tc.tile_pool(name="consts", bufs=1))
            wpool = stack.enter_context(tc.tile_pool(name="w", bufs=4))
            pp = stack.enter_context(tc.tile_pool(name="psum", bufs=8, space="PSUM"))
            acts = stack.enter_context(tc.tile_pool(name="acts", bufs=1))

            biases_sb = cpool.tile([128, cmap["biases"].shape[1]], F32, tag="biases")
            nc.sync.dma_start(biases_sb[:], ext["biases"][:])
            fcw_sb = cpool.tile([128, 4, 10], F32, tag="fcw")
            nc.sync.dma_start(fcw_sb[:], ext["fcw"][:])
            fbank_sb = cpool.tile([128, 4, 64], BF, tag="fbank")
            nc.sync.dma_start(fbank_sb[:], ext["fbank"][:])

            z = cpool.tile([128, B, 407], BF, tag="z")         # conv1 input, W-padded, paired
            x_l1 = acts.tile([128, 1, B, 18, 103], BF, tag="xl1")  # post-maxpool, padded, paired

            def bias_ap(key, cot, pm):
                return biases_sb[0:pm, cols[key] + cot: cols[key] + cot + 1]

            # ---------------- frontend ----------------
            with tc.tile_pool(name="fe1", bufs=1) as fe1, \
                 tc.tile_pool(name="few", bufs=4) as few:
                warm = fe1.tile([1, 8], F32, tag="warm")
                warm_d = nc.dram_tensor("warm_sink", [1, 8], F32)
                nc.vector.memset(warm[:], 1.0)
                for wi, fn in enumerate((AF.Square, AF.Sqrt, AF.Ln, AF.Exp,
                                         AF.Relu, AF.Identity)):
                    nc.scalar.activation(warm[:, wi:wi + 1], warm[:, wi:wi + 1], fn)
                nc.sync.dma_start(warm_d[:], warm[:])  # keep warm-up live past DCE
                frames_sb = fe1.tile([128, 8, B * T], BF, tag="frames")
                for b in range(B):
                    nc.sync.dma_start(frames_sb[:, :, b * T:(b + 1) * T],
                                      ext["frames"][:, :, b * T:(b + 1) * T])
                dftm_sb = fe1.tile([128, 8, 2, NF], BF, tag="dftm")
                nc.sync.dma_start(dftm_sb[:], ext["dftm"][:])
                mag = fe1.tile([128, 4, B * T], BF, tag="mag")
                mel = fe1.tile([64, B * T], F32, tag="mel")
                A = fe1.tile([64, B * T], F32, tag="A")
                Bt = fe1.tile([64, B * T], F32, tag="Bt")
                scanc = fe1.tile([64, T], F32, tag="scanc")
                nc.vector.memset(scanc[:], 1.0 - S)

                # DFT + magnitude: sample-major so mel/PCEN pipeline per sample
                for b in range(B):
                    for mt in range(4):
                        ps_re = pp.tile([128, T], F32, tag="ps")
                        ps_im = pp.tile([128, T], F32, tag="ps")
                        for c in range(8):
                            rhs = frames_sb[:, c, b * T:(b + 1) * T]
                            nc.tensor.matmul(ps_re[:], dftm_sb[:, c, 0, mt * 128:(mt + 1) * 128],
                                             rhs, start=(c == 0), stop=(c == 7))
                            nc.tensor.matmul(ps_im[:], dftm_sb[:, c, 1, mt * 128:(mt + 1) * 128],
                                             rhs, start=(c == 0), stop=(c == 7))
                        sq1 = few.tile([128, T], F32, tag="sq")
                        sq2 = few.tile([128, T], F32, tag="sq")
                        nc.scalar.activation(sq1[:], ps_re[:], AF.Square)
                        nc.scalar.activation(sq2[:], ps_im[:], AF.Square)
                        nc.vector.tensor_add(sq1[:], sq1[:], sq2[:])
                        nc.scalar.activation(mag[:, mt, b * T:(b + 1) * T], sq1[:], AF.Sqrt)

                # mel projection: [64, T] per sample
                for b in range(B):
                    ps = pp.tile([64, T], F32, tag="ps")
                    for ct in range(4):
                        nc.tensor.matmul(ps[:], fbank_sb[:, ct, :],
                                         mag[:, ct, b * T:(b + 1) * T],
                                         start=(ct == 0), stop=(ct == 3))
                    nc.scalar.activation(mel[:, b * T:(b + 1) * T], ps[:], AF.Copy)

                # PCEN
                for b in range(B):
                    sl = slice(b * T, (b + 1) * T)
                    nc.vector.tensor_scalar_mul(A[:, sl], mel[:, sl], S)
                    nc.vector.tensor_copy(A[:, b * T:b * T + 1], mel[:, b * T:b * T + 1])
                    nc.vector.tensor_tensor_scan(
                        Bt[:, sl], scanc[:], A[:, sl],
                        mel[:, b * T:b * T + 1], ALU.mult, ALU.add)
                nc.vector.memset(z[:, :, 0:3], 0.0)
                nc.vector.memset(z[:, :, 404:407], 0.0)
                for b in range(B):
                    sl = slice(b * T, (b + 1) * T)
                    nc.scalar.activation(A[:, sl], Bt[:, sl], AF.Ln, bias=EPS)
                    nc.scalar.activation(Bt[:, sl], A[:, sl], AF.Exp, scale=-meta["alpha"])
                    nc.vector.tensor_mul(A[:, sl], mel[:, sl], Bt[:, sl])
                    nc.scalar.activation(Bt[:, sl], A[:, sl], AF.Sqrt, bias=meta["delta"])
                    nc.vector.tensor_scalar(
                        z[0:64, b, 3:404], Bt[:, sl],
                        meta["s0"], meta["c0"], ALU.mult, ALU.add)
                    # paired copy: rows 64:128 = rows 0:64 shifted left by 1
                    nc.sync.dma_start(z[64:128, b, 0:406], z[0:64, b, 1:407])

            if dbg == 1:
                nc.sync.dma_start(dbg_ext[:], z[:])

            # ---------------- conv1 + maxpool ----------------
            with tc.tile_pool(name="fe2", bufs=1) as fe2, \
                 tc.tile_pool(name="fe2w", bufs=8) as fe2w:
                w1_t = []
                for j in range(4):
                    t_ = fe2.tile([128, 2048], BF, tag=f"w1_{j}")
                    nc.scalar.dma_start(t_[:], ext["w1m"][j])
                    w1_t.append(t_)
                y1p = fe2.tile([64, B, 34, 203], BF, tag="y1p")
                nc.vector.memset(y1p[:, :, 0:1, :], 0.0)
                nc.vector.memset(y1p[:, :, 33:34, :], 0.0)
                nc.vector.memset(y1p[:, :, :, 0:1], 0.0)
                nc.vector.memset(y1p[:, :, :, 202:203], 0.0)
                tv = fe2.tile([64, B, 16, 203], BF, tag="tv")
                tw = fe2.tile([64, B, 16, 101], BF, tag="tw")
                nc.vector.memset(x_l1[:, 0, :, 0:1, :], 0.0)
                nc.vector.memset(x_l1[:, 0, :, 17:18, :], 0.0)
                nc.vector.memset(x_l1[:, 0, :, :, 0:1], 0.0)
                nc.vector.memset(x_l1[:, 0, :, :, 102:103], 0.0)
                for bp in (0, 2):
                    for mt in range(16):
                        ps = pp.tile([128, 2, 201], F32, tag="ps")
                        for j in range(4):
                            nc.tensor.matmul(ps[:], w1_t[j][:, mt * 128:(mt + 1) * 128],
                                             z[:, bp:bp + 2, 2 * j:2 * j + 401:2],
                                             start=(j == 0), stop=(j == 3))
                        t1 = fe2w.tile([128, 2, 201], BF, tag="c1t")
                        nc.scalar.activation(t1[:], ps[:], AF.Relu, bias=bias_ap("conv1", mt, 128))
                        nc.sync.dma_start(
                            y1p[4 * mt:4 * mt + 4, bp + 0, 1:33, 1:202], t1[:, 0, :])
                        nc.scalar.dma_start(
                            y1p[4 * mt:4 * mt + 4, bp + 1, 1:33, 1:202], t1[:, 1, :])
                    for b in (bp, bp + 1):
                        # maxpool 3x3 s2 p1 (inputs >= 0; zero pad is safe)
                        nc.vector.tensor_max(tv[:, b], y1p[:, b, 0:32:2, :], y1p[:, b, 1:33:2, :])
                        nc.vector.tensor_max(tv[:, b], tv[:, b], y1p[:, b, 2:34:2, :])
                        nc.vector.tensor_max(tw[:, b], tv[:, b, :, 0:202:2], tv[:, b, :, 1:203:2])
                        nc.vector.tensor_max(x_l1[0:64, 0, b, 1:17, 1:102], tw[:, b], tv[:, b, :, 2:203:2])
                        nc.sync.dma_start(x_l1[64:128, 0, b, :, 0:102], x_l1[0:64, 0, b, :, 1:103])

            if dbg == 2:
                nc.sync.dma_start(dbg_ext[:], x_l1[:])

            # ---------------- ResNet layers ----------------
            def do_conv(X, cin, cout, stride, ksz, Ho, Wo, nts, wname, writer,
                        paired=False):
                """X: padded input tile [P, CT, B, Hp, Wp] (pad=1 iff ksz==3).
                nts: list of ('all'|b, oh0, ohn). writer(psum, cot, nt).
                paired: cin=64, X has W-shifted copy on partitions 64:128;
                weight slots s=kh*2+j cover kw={2j, 2j+1}."""
                P = 128 if paired else min(cin, 128)
                CT = (cin + 127) // 128
                PM = min(cout, 128)
                COT = (cout + 127) // 128
                KK = 6 if paired else ksz * ksz
                off = 0 if ksz == 3 else 1  # 1x1 conv reads interior of padded input
                psums = {}
                for ct in range(CT):
                    wt = wpool.tile([P, KK, cout], BF, tag="w")
                    nc.scalar.dma_start(wt[:], ext[wname][ct])
                    for cot in range(COT):
                        for ni, nt in enumerate(nts):
                            bsel, oh0, ohn = nt
                            if ct == 0:
                                shape = [PM, B, ohn, Wo] if bsel == "all" else [PM, ohn, Wo]
                                psums[(cot, ni)] = pp.tile(shape, F32, tag="ps", name="cps")
                            ps = psums[(cot, ni)]
                            for kk in range(KK):
                                if paired:
                                    kh, j = divmod(kk, 2)
                                    w0 = 2 * j
                                else:
                                    kh, kw = divmod(kk, ksz)
                                    w0 = kw + off
                                h0 = kh + off + stride * oh0
                                hsl = slice(h0, h0 + stride * (ohn - 1) + 1, stride)
                                wsl = slice(w0, w0 + stride * (Wo - 1) + 1, stride)
                                if bsel == "all":
                                    rhs = X[0:P, ct, :, hsl, wsl]
                                else:
                                    rhs = X[0:P, ct, bsel, hsl, wsl]
                                nc.tensor.matmul(
                                    ps[:], wt[:, kk, cot * 128:cot * 128 + PM], rhs,
                                    start=(ct == 0 and kk == 0),
                                    stop=(ct == CT - 1 and kk == KK - 1))
                            if ct == CT - 1:
                                writer(ps, cot, nt)

            def interior(Xt, cot, nt, Ho, Wo, pad=1, pm=None):
                bsel, oh0, ohn = nt
                pm = Xt.shape[0] if pm is None else pm
                if bsel == "all":
                    return Xt[0:pm, cot, :, pad + oh0:pad + oh0 + ohn, pad:pad + Wo]
                return Xt[0:pm, cot, bsel, pad + oh0:pad + oh0 + ohn, pad:pad + Wo]

            def relu_writer(dest, key, Ho, Wo, pm):
                def w(ps, cot, nt):
                    nc.scalar.activation(interior(dest, cot, nt, Ho, Wo, pm=pm), ps[:],
                                         AF.Relu, bias=bias_ap(key, cot, pm))
                return w

            def ident_writer(dest, key, Ho, Wo, pm):
                def w(ps, cot, nt):
                    nc.scalar.activation(interior(dest, cot, nt, Ho, Wo, pad=0), ps[:],
                                         AF.Identity, bias=bias_ap(key, cot, pm))
                return w

            def res_writer(dest, key, scget, Ho, Wo, pm):
                def w(ps, cot, nt):
                    nc.vector.tensor_add(ps[:], ps[:], scget(cot, nt))
                    nc.scalar.activation(interior(dest, cot, nt, Ho, Wo, pm=pm), ps[:],
                                         AF.Relu, bias=bias_ap(key, cot, pm))
                return w

            def memset_border(Xt, P, CT, Hp, Wp):
                for ct in range(CT):
                    nc.vector.memset(Xt[0:P, ct, :, 0:1, :], 0.0)
                    nc.vector.memset(Xt[0:P, ct, :, Hp - 1:Hp, :], 0.0)
                    nc.vector.memset(Xt[0:P, ct, :, :, 0:1], 0.0)
                    nc.vector.memset(Xt[0:P, ct, :, :, Wp - 1:Wp], 0.0)

            x = x_l1
            cin = 64
            lpools = [stack.enter_context(tc.tile_pool(name=f"l{li}", bufs=1))
                      for li in range(4)]
            for li, (cout, nb) in enumerate(CFG):
                Ho, Wo = GEOM[li + 1]
                Hp, Wp = Ho + 2, Wo + 2
                P = min(cout, 128)
                COT = (cout + 127) // 128
                if li == 0:
                    nts = [(b, o, n) for b in range(B) for o, n in
                           [(0, 5), (5, 5), (10, 5), (15, 1)]]
                elif li == 1:
                    nts = [(b, 0, Ho) for b in range(B)]
                else:
                    nts = [("all", 0, Ho)]
                lp = lpools[li]
                PT = 128 if li == 0 else P  # layer-1 tiles carry the paired copy
                for bi in range(nb):
                    stride = 2 if (li > 0 and bi == 0) else 1
                    y = lp.tile([PT, COT, B, Hp, Wp], BF, tag=f"y{li}")
                    xo = lp.tile([PT, COT, B, Hp, Wp], BF, tag=f"x{li}_{bi % 2}")
                    if bi == 0:
                        memset_border(y, PT, COT, Hp, Wp)
                    if bi < 2:
                        memset_border(xo, PT, COT, Hp, Wp)

                    def cat(Xt, Hp=Hp, Wp=Wp):
                        for b in range(B):
                            nc.sync.dma_start(Xt[64:128, 0, b, :, 0:Wp - 1],
                                              Xt[0:64, 0, b, :, 1:Wp])

                    k1 = f"w_{li}_{bi}_1"
                    do_conv(x, cin, cout, stride, 3, Ho, Wo, nts, k1,
                            relu_writer(y, k1, Ho, Wo, P), paired=(k1 in PAIRED))
                    if li == 0:
                        cat(y)
                    if stride != 1 or cin != cout:
                        sc = lp.tile([P, COT, B, Ho, Wo], BF, tag=f"sc{li}")
                        kd = f"w_{li}_{bi}_d"
                        do_conv(x, cin, cout, stride, 1, Ho, Wo, nts, kd,
                                ident_writer(sc, kd, Ho, Wo, P))
                        scget = lambda cot, nt, sc=sc: interior(sc, cot, nt, Ho, Wo, pad=0)
                    else:
                        scget = lambda cot, nt, x=x: interior(x, cot, nt, Ho, Wo, pm=P)
                    k2 = f"w_{li}_{bi}_2"
                    do_conv(y, cout, cout, 1, 3, Ho, Wo, nts, k2,
                            res_writer(xo, k2, scget, Ho, Wo, P), paired=(k2 in PAIRED))
                    if li == 0:
                        cat(xo)
                    x = xo
                    cin = cout

            # ---------------- avgpool + fc ----------------
            feat = cpool.tile([128, 4, B], F32, tag="feat")
            for ct in range(4):
                nc.vector.tensor_reduce(feat[:, ct, :], x[:, ct, :, 1:3, 1:14],
                                        AX.XY, ALU.add)
            psf = pp.tile([10, B], F32, tag="ps")
            for ct in range(4):
                nc.tensor.matmul(psf[:], fcw_sb[:, ct, :], feat[:, ct, :],
                                 start=(ct == 0), stop=(ct == 3))
            out_sb = cpool.tile([10, B], F32, tag="outsb")
            nc.scalar.activation(out_sb[:], psf[:], AF.Identity,
                                 bias=bias_ap("fc", 0, 10))
            nc.sync.dma_start(out_ext[:], out_sb[:])

    nc.compile()
    return nc


# ---------------------------------------------------------------- entry
def kernel(x, params):
    x = np.asarray(x, np.float32)
    cmap, meta = host_prep(params)
    dbg = int(os.environ.get("AK_DBG", "0"))
    nc = build_graph(cmap, meta, dbg=dbg)
    in_maps = []
    for c in range(NCORES):
        m = {"frames": frames_for_shard(x[c * B:(c + 1) * B])}
        for k, v in cmap.items():
            m[k] = np.asarray(v)
        in_maps.append(m)
    res = run_bass_kernel_spmd(nc, in_maps, list(range(NCORES)),
                               trace=bool(int(os.environ.get("AK_TRACE", "0"))))
    outs = [res.results[c]["out"].T for c in range(NCORES)]  # [4, 10] each
    if dbg:
        kernel.dbg = [res.results[c].get("dbg") for c in range(NCORES)]
    kernel.exec_time_ns = res.exec_time_ns
    return np.concatenate(outs, axis=0).astype(np.float32)


# revision 16
# speedup vs baseline: 1.1112x; 1.1112x over previous
"""Trainium2 Bass kernel for AudioResNetPCEN.

Data-parallel over 8 NeuronCores: 4 samples/core. Per core:
  frames (host im2col, bf16) -> DFT via matmul (re/im) -> |.| -> mel matmul
  -> PCEN (IIR via DVE tensor_tensor_scan, exact recurrence) -> input BN
  -> conv1 7x7/s2 via host-built banded matrices (contraction over mel axis)
  -> maxpool 3x3/s2 (shifted-view DVE max)
  -> ResNet34 basic blocks: 3x3 convs as 9 shifted 1x1 matmuls, BN folded
     into weights on host, bias+ReLU fused into the PSUM->SBUF activation
  -> avgpool (tensor_reduce) -> fc matmul -> [10, 4] out per core.
"""
import os
import sys
import numpy as np

sys.path.insert(0, "/opt/trn_rl_repo")

import ml_dtypes
from concourse import bass, bacc, tile, mybir
from concourse.bass_utils import run_bass_kernel_spmd

BF16 = ml_dtypes.bfloat16

SR = 32000; NFFT = 1024; HOP = 320; NMELS = 64
EPS = 1e-6; BN_EPS = 1e-5
S = HOP / (SR * 0.06)
T = 401          # frames per sample
B = 4            # samples per core
NCORES = 8
NF = 512         # freq bins used (bin 512 has zero mel weight)

F32 = mybir.dt.float32
BF = mybir.dt.bfloat16
AF = mybir.ActivationFunctionType
ALU = mybir.AluOpType
AX = mybir.AxisListType

# ResNet34 config: (cout, nblocks) per layer; spatial geometry per layer input
CFG = [(64, 3), (128, 4), (256, 6), (512, 3)]
# geometry AFTER each layer: layer i blocks operate at OUT spatial size
GEOM = {1: (16, 101), 2: (8, 51), 3: (4, 26), 4: (2, 13)}


# ---------------------------------------------------------------- host prep
def _hz_to_mel(f):
    return 2595.0 * np.log10(1.0 + f / 700.0)


def _mel_to_hz(m):
    return 700.0 * (10.0 ** (m / 2595.0) - 1.0)


def mel_fbank_np():
    n_freqs = NFFT // 2 + 1
    all_freqs = np.linspace(0.0, SR // 2, n_freqs)
    m_pts = np.linspace(_hz_to_mel(20.0), _hz_to_mel(16000.0), NMELS + 2)
    f_pts = _mel_to_hz(m_pts)
    f_diff = f_pts[1:] - f_pts[:-1]
    slopes = f_pts[None, :] - all_freqs[:, None]
    down = -slopes[:, :-2] / f_diff[:-1]
    up = slopes[:, 2:] / f_diff[1:]
    return np.maximum(0.0, np.minimum(down, up)).astype(np.float32)  # [513, 64]


def build_dft():
    n = np.arange(NFFT)
    win = 0.5 * (1.0 - np.cos(2.0 * np.pi * n / NFFT))
    k = np.arange(NF)
    ang = 2.0 * np.pi * np.outer(n, k) / NFFT
    Cre = (win[:, None] * np.cos(ang)).astype(np.float32)
    Cim = (win[:, None] * np.sin(ang)).astype(np.float32)
    # dftm[p, c, ri, k] = C[128c+p, k]
    dftm = np.stack([Cre.reshape(8, 128, NF), Cim.reshape(8, 128, NF)], axis=2)
    return np.ascontiguousarray(dftm.transpose(1, 0, 2, 3)).astype(BF16)  # [128,8,2,512]


def conv1_mats(w1f):
    """w1f [64,1,7,7] folded -> paired [4, 128, 2048]: pair j rows 0:64 = B_{2j},
    rows 64:128 = B_{2j+1} (zero for j=3); B_kw[ih, oc*32+oh] = w1f[oc,0,kh,kw]."""
    bm = np.zeros((7, 64, 2048), np.float32)
    kh = np.arange(7)
    for kw in range(7):
        for oc in range(64):
            for oh in range(32):
                ih = 2 * oh + kh - 3
                m = (ih >= 0) & (ih < 64)
                bm[kw, ih[m], oc * 32 + oh] = w1f[oc, 0, kh[m], kw]
    out = np.zeros((4, 128, 2048), np.float32)
    for j in range(4):
        out[j, :64] = bm[2 * j]
        if 2 * j + 1 < 7:
            out[j, 64:] = bm[2 * j + 1]
    return out.astype(BF16)


def fold(w, g):
    return (np.asarray(w, np.float32)
            * (np.asarray(g, np.float32) / np.sqrt(1.0 + BN_EPS))[:, None, None, None])


def conv_names():
    names = []
    cin = 64
    for li, (c, n) in enumerate(CFG):
        for bi in range(n):
            stride = 2 if (li > 0 and bi == 0) else 1
            names.append((f"w_{li}_{bi}_1", cin, c, 3, stride))
            names.append((f"w_{li}_{bi}_2", c, c, 3, 1))
            if stride != 1 or cin != c:
                names.append((f"w_{li}_{bi}_d", cin, c, 1, stride))
            cin = c
    return names


def pack_conv(wf):
    """wf [Cout, Cin, k, k] -> [CT, 128 (or cin), KK, Cout] array"""
    co, ci, k, _ = wf.shape
    P = min(ci, 128)
    CT = (ci + 127) // 128
    out = np.zeros((CT, P, k * k, co), np.float32)
    for ct in range(CT):
        sl = wf[:, ct * 128:(ct + 1) * 128]  # [co, P, k, k]
        out[ct] = sl.reshape(co, P, k * k).transpose(1, 2, 0)
    return out.astype(BF16)


def pack_conv_paired(wf):
    """wf [Cout, 64, 3, 3] -> [1, 128, 6, Cout]; slot s=kh*2+j:
    j=0: rows 0:64 tap kw=0, rows 64:128 tap kw=1; j=1: rows 0:64 tap kw=2."""
    co = wf.shape[0]
    assert wf.shape[1] == 64 and wf.shape[2] == 3
    out = np.zeros((1, 128, 6, co), np.float32)
    for kh in range(3):
        out[0, :64, kh * 2 + 0] = wf[:, :, kh, 0].T
        out[0, 64:, kh * 2 + 0] = wf[:, :, kh, 1].T
        out[0, :64, kh * 2 + 1] = wf[:, :, kh, 2].T
    return out.astype(BF16)


PAIRED = set()
for _bi in range(3):
    PAIRED.add(f"w_0_{_bi}_1"); PAIRED.add(f"w_0_{_bi}_2")
PAIRED.add("w_1_0_1")


def host_prep(params):
    """-> (const_map {name: np.ndarray}, meta)"""
    p = params
    cmap = {}
    cmap["dftm"] = build_dft()
    fb = mel_fbank_np()
    assert abs(fb[512]).max() == 0.0
    fbank = np.zeros((128, 4, 64), np.float32)
    for ct in range(4):
        fbank[:, ct] = fb[ct * 128:(ct + 1) * 128]
    cmap["fbank"] = fbank.astype(BF16)

    w1f = fold(p["conv1"], p["g1"])
    cmap["w1m"] = conv1_mats(w1f)

    # biases: one fp32 [128, NB] array; column map
    cols = {}
    bias_cols = []
    b1 = np.asarray(p["b1"], np.float32)
    for mt in range(16):
        col = np.zeros(128, np.float32)
        for pp in range(128):
            col[pp] = b1[4 * mt + pp // 32]
        bias_cols.append(col)
    cols["conv1"] = 0

    def add_bias(name, bias):
        cols[name] = len(bias_cols)
        co = bias.shape[0]
        for cot in range((co + 127) // 128):
            col = np.zeros(128, np.float32)
            seg = bias[cot * 128:(cot + 1) * 128]
            col[:len(seg)] = seg
            bias_cols.append(col)

    cin = 64
    for li, (c, n) in enumerate(CFG):
        for bi in range(n):
            stride = 2 if (li > 0 and bi == 0) else 1
            bp = p["layers"][li][bi]
            pk1 = pack_conv_paired if f"w_{li}_{bi}_1" in PAIRED else pack_conv
            pk2 = pack_conv_paired if f"w_{li}_{bi}_2" in PAIRED else pack_conv
            cmap[f"w_{li}_{bi}_1"] = pk1(fold(bp["w1"], bp["g1"]))
            add_bias(f"w_{li}_{bi}_1", np.asarray(bp["b1"], np.float32))
            cmap[f"w_{li}_{bi}_2"] = pk2(fold(bp["w2"], bp["g2"]))
            add_bias(f"w_{li}_{bi}_2", np.asarray(bp["b2"], np.float32))
            if stride != 1 or cin != c:
                cmap[f"w_{li}_{bi}_d"] = pack_conv(fold(bp["wd"], bp["gd"]))
                add_bias(f"w_{li}_{bi}_d", np.asarray(bp["bd"], np.float32))
            cin = c
    # fc
    fcb = np.zeros(128, np.float32)
    fcb[:10] = np.asarray(p["fc_b"], np.float32)
    cols["fc"] = len(bias_cols)
    bias_cols.append(fcb)
    cmap["biases"] = np.stack(bias_cols, axis=1).astype(np.float32)  # [128, NB]

    fcw = np.zeros((128, 4, 10), np.float32)
    w = np.asarray(p["fc_w"], np.float32) / 26.0  # fold avgpool 1/(2*13)
    for ct in range(4):
        fcw[:, ct] = w[:, ct * 128:(ct + 1) * 128].T
    cmap["fcw"] = fcw

    # PCEN / input-bn scalars
    alpha = float(np.clip(np.asarray(p["alpha"]), 0.01, 0.99))
    delta = float(abs(np.asarray(p["delta"])) + EPS)
    r = float(np.clip(np.asarray(p["r"]), 0.01, 1.0))
    assert abs(r - 0.5) < 1e-6, "kernel assumes r=0.5 (sqrt)"
    s0 = float(np.asarray(p["inbn_g"])[0] / np.sqrt(1.0 + BN_EPS))
    c0 = float(np.asarray(p["inbn_b"])[0] - (delta ** r) * s0)
    meta = dict(alpha=alpha, delta=delta, s0=s0, c0=c0, cols=cols)
    return cmap, meta


def frames_for_shard(xs):
    """xs [B, 128000] fp32 -> [128, 8, B*T] bf16 (frames_T[p,c,(b,t)] = xp[b, 320t+128c+p])"""
    xp = np.pad(xs, ((0, 0), (512, 512)), mode="reflect")
    sw = np.lib.stride_tricks.sliding_window_view(xp, NFFT, axis=1)  # [B, L-1023, 1024]
    fr = sw[:, ::HOP][:, :T]                    # [B, T, 1024]
    fr = fr.transpose(2, 0, 1).reshape(8, 128, B * T)  # [c, p, bt]
    return np.ascontiguousarray(fr.transpose(1, 0, 2)).astype(BF16)


# ---------------------------------------------------------------- graph
def build_graph(cmap, meta, dbg=0):
    nc = bacc.Bacc("TRN2", target_bir_lowering=False, debug=False,
                   num_devices=NCORES)
    cols = meta["cols"]

    for v in (float(EPS), float(meta["delta"])):
        if (F32, v) not in nc.const_aps.aps:
            t_ = nc.alloc_sbuf_tensor(f"const-f32-{v}", [128, 1], F32)
            nc.gpsimd.memset(t_.ap(), v)
            nc.const_aps.aps[(F32, v)] = t_.ap()

    ext = {}
    ext["frames"] = nc.dram_tensor("frames", [128, 8, B * T], BF, kind="ExternalInput")
    for name, arr in cmap.items():
        dt = BF if arr.dtype == BF16 else F32
        ext[name] = nc.dram_tensor(name, list(arr.shape), dt, kind="ExternalInput")
    out_ext = nc.dram_tensor("out", [10, B], F32, kind="ExternalOutput")
    if dbg == 1:
        dbg_ext = nc.dram_tensor("dbg", [64, B, 407], BF, kind="ExternalOutput")
    elif dbg == 2:
        dbg_ext = nc.dram_tensor("dbg", [64, 1, B, 18, 103], BF, kind="ExternalOutput")
    elif dbg == 3:
        dbg_ext = nc.dram_tensor("dbg", [64, 1, B, 18, 103], BF, kind="ExternalOutput")

    with tile.TileContext(nc) as tc:
        import contextlib
        stack = contextlib.ExitStack()
        with stack:
            cpool = stack.enter_context(tc.tile_pool(name="consts", bufs=1))
            wpool = stack.enter_context(tc.tile_pool(name="w", bufs=4))
            pp = stack.enter_context(tc.tile_pool(name="psum", bufs=8, space="PSUM"))
            acts = stack.enter_context(tc.tile_pool(name="acts", bufs=1))

            biases_sb = cpool.tile([128, cmap["biases"].shape[1]], F32, tag="biases")
            nc.sync.dma_start(biases_sb[:], ext["biases"][:])
            fcw_sb = cpool.tile([128, 4, 10], F32, tag="fcw")
            nc.sync.dma_start(fcw_sb[:], ext["fcw"][:])
            fbank_sb = cpool.tile([128, 4, 64], BF, tag="fbank")
            nc.sync.dma_start(fbank_sb[:], ext["fbank"][:])

            z = cpool.tile([128, B, 407], BF, tag="z")         # conv1 input, W-padded, paired
            x_l1 = acts.tile([128, 1, B, 18, 103], BF, tag="xl1")  # post-maxpool, padded, paired

            def bias_ap(key, cot, pm):
                return biases_sb[0:pm, cols[key] + cot: cols[key] + cot + 1]

            # ---------------- frontend ----------------
            with tc.tile_pool(name="fe1", bufs=1) as fe1, \
                 tc.tile_pool(name="few", bufs=4) as few:
                warm = fe1.tile([1, 8], F32, tag="warm")
                warm_d = nc.dram_tensor("warm_sink", [1, 8], F32)
                nc.vector.memset(warm[:], 1.0)
                for wi, fn in enumerate((AF.Square, AF.Sqrt, AF.Ln, AF.Exp,
                                         AF.Relu, AF.Identity)):
                    nc.scalar.activation(warm[:, wi:wi + 1], warm[:, wi:wi + 1], fn)
                nc.sync.dma_start(warm_d[:], warm[:])  # keep warm-up live past DCE
                frames_sb = fe1.tile([128, 8, B * T], BF, tag="frames")
                for b in range(B):
                    nc.sync.dma_start(frames_sb[:, :, b * T:(b + 1) * T],
                                      ext["frames"][:, :, b * T:(b + 1) * T])
                dftm_sb = fe1.tile([128, 8, 2, NF], BF, tag="dftm")
                nc.sync.dma_start(dftm_sb[:], ext["dftm"][:])
                mag = fe1.tile([128, 4, B * T], BF, tag="mag")
                mel = fe1.tile([64, B * T], F32, tag="mel")
                A = fe1.tile([64, B * T], F32, tag="A")
                Bt = fe1.tile([64, B * T], F32, tag="Bt")
                scanc = fe1.tile([64, T], F32, tag="scanc")
                nc.vector.memset(scanc[:], 1.0 - S)

                # DFT + magnitude: sample-major so mel/PCEN pipeline per sample
                for b in range(B):
                    for mt in range(4):
                        ps_re = pp.tile([128, T], F32, tag="ps")
                        ps_im = pp.tile([128, T], F32, tag="ps")
                        for c in range(8):
                            rhs = frames_sb[:, c, b * T:(b + 1) * T]
                            nc.tensor.matmul(ps_re[:], dftm_sb[:, c, 0, mt * 128:(mt + 1) * 128],
                                             rhs, start=(c == 0), stop=(c == 7))
                            nc.tensor.matmul(ps_im[:], dftm_sb[:, c, 1, mt * 128:(mt + 1) * 128],
                                             rhs, start=(c == 0), stop=(c == 7))
                        sq1 = few.tile([128, T], F32, tag="sq")
                        sq2 = few.tile([128, T], F32, tag="sq")
                        nc.scalar.activation(sq1[:], ps_re[:], AF.Square)
                        nc.scalar.activation(sq2[:], ps_im[:], AF.Square)
                        nc.vector.tensor_add(sq1[:], sq1[:], sq2[:])
                        nc.scalar.activation(mag[:, mt, b * T:(b + 1) * T], sq1[:], AF.Sqrt)

                # mel projection: [64, T] per sample
                for b in range(B):
                    ps = pp.tile([64, T], F32, tag="ps")
                    for ct in range(4):
                        nc.tensor.matmul(ps[:], fbank_sb[:, ct, :],
                                         mag[:, ct, b * T:(b + 1) * T],
                                         start=(ct == 0), stop=(ct == 3))
                    nc.scalar.activation(mel[:, b * T:(b + 1) * T], ps[:], AF.Copy)

                # PCEN
                for b in range(B):
                    sl = slice(b * T, (b + 1) * T)
                    nc.vector.tensor_scalar_mul(A[:, sl], mel[:, sl], S)
                    nc.vector.tensor_copy(A[:, b * T:b * T + 1], mel[:, b * T:b * T + 1])
                    nc.vector.tensor_tensor_scan(
                        Bt[:, sl], scanc[:], A[:, sl],
                        mel[:, b * T:b * T + 1], ALU.mult, ALU.add)
                nc.vector.memset(z[:, :, 0:3], 0.0)
                nc.vector.memset(z[:, :, 404:407], 0.0)
                for b in range(B):
                    sl = slice(b * T, (b + 1) * T)
                    nc.scalar.activation(A[:, sl], Bt[:, sl], AF.Ln, bias=EPS)
                    nc.scalar.activation(Bt[:, sl], A[:, sl], AF.Exp, scale=-meta["alpha"])
                    nc.vector.tensor_mul(A[:, sl], mel[:, sl], Bt[:, sl])
                    nc.scalar.activation(Bt[:, sl], A[:, sl], AF.Sqrt, bias=meta["delta"])
                    nc.vector.tensor_scalar(
                        z[0:64, b, 3:404], Bt[:, sl],
                        meta["s0"], meta["c0"], ALU.mult, ALU.add)
                    # paired copy: rows 64:128 = rows 0:64 shifted left by 1
                    nc.sync.dma_start(z[64:128, b, 0:406], z[0:64, b, 1:407])

            if dbg == 1:
                nc.sync.dma_start(dbg_ext[:], z[:])

            # ---------------- conv1 + maxpool ----------------
            with tc.tile_pool(name="fe2", bufs=1) as fe2, \
                 tc.tile_pool(name="fe2w", bufs=16) as fe2w:
                w1_t = []
                for j in range(4):
                    t_ = fe2.tile([128, 2048], BF, tag=f"w1_{j}")
                    nc.scalar.dma_start(t_[:], ext["w1m"][j])
                    w1_t.append(t_)
                y1p = fe2.tile([64, B, 34, 203], BF, tag="y1p")
                nc.vector.memset(y1p[:, :, 0:1, :], 0.0)
                nc.vector.memset(y1p[:, :, 33:34, :], 0.0)
                nc.vector.memset(y1p[:, :, :, 0:1], 0.0)
                nc.vector.memset(y1p[:, :, :, 202:203], 0.0)
                tv = fe2.tile([64, B, 16, 203], BF, tag="tv")
                tw = fe2.tile([64, B, 16, 101], BF, tag="tw")
                nc.vector.memset(x_l1[:, 0, :, 0:1, :], 0.0)
                nc.vector.memset(x_l1[:, 0, :, 17:18, :], 0.0)
                nc.vector.memset(x_l1[:, 0, :, :, 0:1], 0.0)
                nc.vector.memset(x_l1[:, 0, :, :, 102:103], 0.0)
                for bp in (0, 2):
                    for mt in range(16):
                        ps = pp.tile([128, 2, 201], F32, tag="ps")
                        for j in range(4):
                            nc.tensor.matmul(ps[:], w1_t[j][:, mt * 128:(mt + 1) * 128],
                                             z[:, bp:bp + 2, 2 * j:2 * j + 401:2],
                                             start=(j == 0), stop=(j == 3))
                        t1 = fe2w.tile([128, 2, 201], BF, tag="c1t")
                        nc.scalar.activation(t1[:], ps[:], AF.Relu, bias=bias_ap("conv1", mt, 128))
                        for j in range(2):
                            nc.sync.dma_start(
                                y1p[4 * mt:4 * mt + 4, bp + j, 1:33, 1:202], t1[:, j, :])
                    for b in (bp, bp + 1):
                        # maxpool 3x3 s2 p1 (inputs >= 0; zero pad is safe)
                        nc.vector.tensor_max(tv[:, b], y1p[:, b, 0:32:2, :], y1p[:, b, 1:33:2, :])
                        nc.vector.tensor_max(tv[:, b], tv[:, b], y1p[:, b, 2:34:2, :])
                        nc.vector.tensor_max(tw[:, b], tv[:, b, :, 0:202:2], tv[:, b, :, 1:203:2])
                        nc.vector.tensor_max(x_l1[0:64, 0, b, 1:17, 1:102], tw[:, b], tv[:, b, :, 2:203:2])
                        nc.sync.dma_start(x_l1[64:128, 0, b, :, 0:102], x_l1[0:64, 0, b, :, 1:103])

            if dbg == 2:
                nc.sync.dma_start(dbg_ext[:], x_l1[:])

            # ---------------- ResNet layers ----------------
            def do_conv(X, cin, cout, stride, ksz, Ho, Wo, nts, wname, writer,
                        paired=False):
                """X: padded input tile [P, CT, B, Hp, Wp] (pad=1 iff ksz==3).
                nts: list of ('all'|b, oh0, ohn). writer(psum, cot, nt).
                paired: cin=64, X has W-shifted copy on partitions 64:128;
                weight slots s=kh*2+j cover kw={2j, 2j+1}."""
                P = 128 if paired else min(cin, 128)
                CT = (cin + 127) // 128
                PM = min(cout, 128)
                COT = (cout + 127) // 128
                KK = 6 if paired else ksz * ksz
                off = 0 if ksz == 3 else 1  # 1x1 conv reads interior of padded input
                psums = {}
                for ct in range(CT):
                    wt = wpool.tile([P, KK, cout], BF, tag="w")
                    nc.scalar.dma_start(wt[:], ext[wname][ct])
                    for cot in range(COT):
                        for ni, nt in enumerate(nts):
                            bsel, oh0, ohn = nt
                            if ct == 0:
                                shape = [PM, B, ohn, Wo] if bsel == "all" else [PM, ohn, Wo]
                                psums[(cot, ni)] = pp.tile(shape, F32, tag="ps", name="cps")
                            ps = psums[(cot, ni)]
                            for kk in range(KK):
                                if paired:
                                    kh, j = divmod(kk, 2)
                                    w0 = 2 * j
                                else:
                                    kh, kw = divmod(kk, ksz)
                                    w0 = kw + off
                                h0 = kh + off + stride * oh0
                                hsl = slice(h0, h0 + stride * (ohn - 1) + 1, stride)
                                wsl = slice(w0, w0 + stride * (Wo - 1) + 1, stride)
                                if bsel == "all":
                                    rhs = X[0:P, ct, :, hsl, wsl]
                                else:
                                    rhs = X[0:P, ct, bsel, hsl, wsl]
                                nc.tensor.matmul(
                                    ps[:], wt[:, kk, cot * 128:cot * 128 + PM], rhs,
                                    start=(ct == 0 and kk == 0),
                                    stop=(ct == CT - 1 and kk == KK - 1))
                            if ct == CT - 1:
                                writer(ps, cot, nt)

            def interior(Xt, cot, nt, Ho, Wo, pad=1, pm=None):
                bsel, oh0, ohn = nt
                pm = Xt.shape[0] if pm is None else pm
                if bsel == "all":
                    return Xt[0:pm, cot, :, pad + oh0:pad + oh0 + ohn, pad:pad + Wo]
                return Xt[0:pm, cot, bsel, pad + oh0:pad + oh0 + ohn, pad:pad + Wo]

            def relu_writer(dest, key, Ho, Wo, pm):
                def w(ps, cot, nt):
                    nc.scalar.activation(interior(dest, cot, nt, Ho, Wo, pm=pm), ps[:],
                                         AF.Relu, bias=bias_ap(key, cot, pm))
                return w

            def ident_writer(dest, key, Ho, Wo, pm):
                def w(ps, cot, nt):
                    nc.scalar.activation(interior(dest, cot, nt, Ho, Wo, pad=0), ps[:],
                                         AF.Identity, bias=bias_ap(key, cot, pm))
                return w

            def res_writer(dest, key, scget, Ho, Wo, pm):
                def w(ps, cot, nt):
                    nc.vector.tensor_add(ps[:], ps[:], scget(cot, nt))
                    nc.scalar.activation(interior(dest, cot, nt, Ho, Wo, pm=pm), ps[:],
                                         AF.Relu, bias=bias_ap(key, cot, pm))
                return w

            def memset_border(Xt, P, CT, Hp, Wp):
                for ct in range(CT):
                    nc.vector.memset(Xt[0:P, ct, :, 0:1, :], 0.0)
                    nc.vector.memset(Xt[0:P, ct, :, Hp - 1:Hp, :], 0.0)
                    nc.vector.memset(Xt[0:P, ct, :, :, 0:1], 0.0)
                    nc.vector.memset(Xt[0:P, ct, :, :, Wp - 1:Wp], 0.0)

            x = x_l1
            cin = 64
            lpools = [stack.enter_context(tc.tile_pool(name=f"l{li}", bufs=1))
                      for li in range(4)]
            for li, (cout, nb) in enumerate(CFG):
                Ho, Wo = GEOM[li + 1]
                Hp, Wp = Ho + 2, Wo + 2
                P = min(cout, 128)
                COT = (cout + 127) // 128
                if li == 0:
                    nts = [(b, o, n) for b in range(B) for o, n in
                           [(0, 5), (5, 5), (10, 5), (15, 1)]]
                elif li == 1:
                    nts = [(b, 0, Ho) for b in range(B)]
                else:
                    nts = [("all", 0, Ho)]
                lp = lpools[li]
                PT = 128 if li == 0 else P  # layer-1 tiles carry the paired copy
                for bi in range(nb):
                    stride = 2 if (li > 0 and bi == 0) else 1
                    y = lp.tile([PT, COT, B, Hp, Wp], BF, tag=f"y{li}")
                    xo = lp.tile([PT, COT, B, Hp, Wp], BF, tag=f"x{li}_{bi % 2}")
                    if bi == 0:
                        memset_border(y, PT, COT, Hp, Wp)
                    if bi < 2:
                        memset_border(xo, PT, COT, Hp, Wp)

                    def cat(Xt, Hp=Hp, Wp=Wp):
                        for b in range(B):
                            nc.sync.dma_start(Xt[64:128, 0, b, :, 0:Wp - 1],
                                              Xt[0:64, 0, b, :, 1:Wp])

                    k1 = f"w_{li}_{bi}_1"
                    do_conv(x, cin, cout, stride, 3, Ho, Wo, nts, k1,
                            relu_writer(y, k1, Ho, Wo, P), paired=(k1 in PAIRED))
                    if li == 0:
                        cat(y)
                    if stride != 1 or cin != cout:
                        sc = lp.tile([P, COT, B, Ho, Wo], BF, tag=f"sc{li}")
                        kd = f"w_{li}_{bi}_d"
                        do_conv(x, cin, cout, stride, 1, Ho, Wo, nts, kd,
                                ident_writer(sc, kd, Ho, Wo, P))
                        scget = lambda cot, nt, sc=sc: interior(sc, cot, nt, Ho, Wo, pad=0)
                    else:
                        scget = lambda cot, nt, x=x: interior(x, cot, nt, Ho, Wo, pm=P)
                    k2 = f"w_{li}_{bi}_2"
                    do_conv(y, cout, cout, 1, 3, Ho, Wo, nts, k2,
                            res_writer(xo, k2, scget, Ho, Wo, P), paired=(k2 in PAIRED))
                    if li == 0:
                        cat(xo)
                    x = xo
                    cin = cout

            # ---------------- avgpool + fc ----------------
            feat = cpool.tile([128, 4, B], F32, tag="feat")
            for ct in range(4):
                nc.vector.tensor_reduce(feat[:, ct, :], x[:, ct, :, 1:3, 1:14],
                                        AX.XY, ALU.add)
            psf = pp.tile([10, B], F32, tag="ps")
            for ct in range(4):
                nc.tensor.matmul(psf[:], fcw_sb[:, ct, :], feat[:, ct, :],
                                 start=(ct == 0), stop=(ct == 3))
            out_sb = cpool.tile([10, B], F32, tag="outsb")
            nc.scalar.activation(out_sb[:], psf[:], AF.Identity,
                                 bias=bias_ap("fc", 0, 10))
            nc.sync.dma_start(out_ext[:], out_sb[:])

    nc.compile()
    return nc


# ---------------------------------------------------------------- entry
def kernel(x, params):
    x = np.asarray(x, np.float32)
    cmap, meta = host_prep(params)
    dbg = int(os.environ.get("AK_DBG", "0"))
    nc = build_graph(cmap, meta, dbg=dbg)
    in_maps = []
    for c in range(NCORES):
        m = {"frames": frames_for_shard(x[c * B:(c + 1) * B])}
        for k, v in cmap.items():
            m[k] = np.asarray(v)
        in_maps.append(m)
    res = run_bass_kernel_spmd(nc, in_maps, list(range(NCORES)),
                               trace=bool(int(os.environ.get("AK_TRACE", "0"))))
    outs = [res.results[c]["out"].T for c in range(NCORES)]  # [4, 10] each
    if dbg:
        kernel.dbg = [res.results[c].get("dbg") for c in range(NCORES)]
    kernel.exec_time_ns = res.exec_time_ns
    return np.concatenate(outs, axis=0).astype(np.float32)


# revision 17
# speedup vs baseline: 1.1490x; 1.0341x over previous
"""Trainium2 Bass kernel for AudioResNetPCEN.

Data-parallel over 8 NeuronCores: 4 samples/core. Per core:
  frames (host im2col, bf16) -> DFT via matmul (re/im) -> |.| -> mel matmul
  -> PCEN (IIR via DVE tensor_tensor_scan, exact recurrence) -> input BN
  -> conv1 7x7/s2 via host-built banded matrices (contraction over mel axis)
  -> maxpool 3x3/s2 (shifted-view DVE max)
  -> ResNet34 basic blocks: 3x3 convs as 9 shifted 1x1 matmuls, BN folded
     into weights on host, bias+ReLU fused into the PSUM->SBUF activation
  -> avgpool (tensor_reduce) -> fc matmul -> [10, 4] out per core.
"""
import os
import sys
import numpy as np

sys.path.insert(0, "/opt/trn_rl_repo")

import ml_dtypes
from concourse import bass, bacc, tile, mybir
from concourse.bass_utils import run_bass_kernel_spmd

BF16 = ml_dtypes.bfloat16

SR = 32000; NFFT = 1024; HOP = 320; NMELS = 64
EPS = 1e-6; BN_EPS = 1e-5
S = HOP / (SR * 0.06)
T = 401          # frames per sample
B = 4            # samples per core
NCORES = 8
NF = 512         # freq bins used (bin 512 has zero mel weight)

F32 = mybir.dt.float32
BF = mybir.dt.bfloat16
AF = mybir.ActivationFunctionType
ALU = mybir.AluOpType
AX = mybir.AxisListType

# ResNet34 config: (cout, nblocks) per layer; spatial geometry per layer input
CFG = [(64, 3), (128, 4), (256, 6), (512, 3)]
# geometry AFTER each layer: layer i blocks operate at OUT spatial size
GEOM = {1: (16, 101), 2: (8, 51), 3: (4, 26), 4: (2, 13)}


# ---------------------------------------------------------------- host prep
def _hz_to_mel(f):
    return 2595.0 * np.log10(1.0 + f / 700.0)


def _mel_to_hz(m):
    return 700.0 * (10.0 ** (m / 2595.0) - 1.0)


def mel_fbank_np():
    n_freqs = NFFT // 2 + 1
    all_freqs = np.linspace(0.0, SR // 2, n_freqs)
    m_pts = np.linspace(_hz_to_mel(20.0), _hz_to_mel(16000.0), NMELS + 2)
    f_pts = _mel_to_hz(m_pts)
    f_diff = f_pts[1:] - f_pts[:-1]
    slopes = f_pts[None, :] - all_freqs[:, None]
    down = -slopes[:, :-2] / f_diff[:-1]
    up = slopes[:, 2:] / f_diff[1:]
    return np.maximum(0.0, np.minimum(down, up)).astype(np.float32)  # [513, 64]


def build_dft():
    n = np.arange(NFFT)
    win = 0.5 * (1.0 - np.cos(2.0 * np.pi * n / NFFT))
    k = np.arange(NF)
    ang = 2.0 * np.pi * np.outer(n, k) / NFFT
    Cre = (win[:, None] * np.cos(ang)).astype(np.float32)
    Cim = (win[:, None] * np.sin(ang)).astype(np.float32)
    # dftm[p, c, ri, k] = C[128c+p, k]
    dftm = np.stack([Cre.reshape(8, 128, NF), Cim.reshape(8, 128, NF)], axis=2)
    return np.ascontiguousarray(dftm.transpose(1, 0, 2, 3)).astype(BF16)  # [128,8,2,512]


def conv1_mats(w1f):
    """w1f [64,1,7,7] folded -> paired [4, 128, 2048]: pair j rows 0:64 = B_{2j},
    rows 64:128 = B_{2j+1} (zero for j=3); B_kw[ih, oc*32+oh] = w1f[oc,0,kh,kw]."""
    bm = np.zeros((7, 64, 2048), np.float32)
    kh = np.arange(7)
    for kw in range(7):
        for oc in range(64):
            for oh in range(32):
                ih = 2 * oh + kh - 3
                m = (ih >= 0) & (ih < 64)
                bm[kw, ih[m], oc * 32 + oh] = w1f[oc, 0, kh[m], kw]
    out = np.zeros((4, 128, 2048), np.float32)
    for j in range(4):
        out[j, :64] = bm[2 * j]
        if 2 * j + 1 < 7:
            out[j, 64:] = bm[2 * j + 1]
    return out.astype(BF16)


def fold(w, g):
    return (np.asarray(w, np.float32)
            * (np.asarray(g, np.float32) / np.sqrt(1.0 + BN_EPS))[:, None, None, None])


def conv_names():
    names = []
    cin = 64
    for li, (c, n) in enumerate(CFG):
        for bi in range(n):
            stride = 2 if (li > 0 and bi == 0) else 1
            names.append((f"w_{li}_{bi}_1", cin, c, 3, stride))
            names.append((f"w_{li}_{bi}_2", c, c, 3, 1))
            if stride != 1 or cin != c:
                names.append((f"w_{li}_{bi}_d", cin, c, 1, stride))
            cin = c
    return names


def pack_conv(wf):
    """wf [Cout, Cin, k, k] -> [CT, 128 (or cin), KK, Cout] array"""
    co, ci, k, _ = wf.shape
    P = min(ci, 128)
    CT = (ci + 127) // 128
    out = np.zeros((CT, P, k * k, co), np.float32)
    for ct in range(CT):
        sl = wf[:, ct * 128:(ct + 1) * 128]  # [co, P, k, k]
        out[ct] = sl.reshape(co, P, k * k).transpose(1, 2, 0)
    return out.astype(BF16)


def pack_conv_paired(wf):
    """wf [Cout, 64, 3, 3] -> [1, 128, 6, Cout]; slot s=kh*2+j:
    j=0: rows 0:64 tap kw=0, rows 64:128 tap kw=1; j=1: rows 0:64 tap kw=2."""
    co = wf.shape[0]
    assert wf.shape[1] == 64 and wf.shape[2] == 3
    out = np.zeros((1, 128, 6, co), np.float32)
    for kh in range(3):
        out[0, :64, kh * 2 + 0] = wf[:, :, kh, 0].T
        out[0, 64:, kh * 2 + 0] = wf[:, :, kh, 1].T
        out[0, :64, kh * 2 + 1] = wf[:, :, kh, 2].T
    return out.astype(BF16)


PAIRED = set()
for _bi in range(3):
    PAIRED.add(f"w_0_{_bi}_1"); PAIRED.add(f"w_0_{_bi}_2")
PAIRED.add("w_1_0_1")


def host_prep(params):
    """-> (const_map {name: np.ndarray}, meta)"""
    p = params
    cmap = {}
    cmap["dftm"] = build_dft()
    fb = mel_fbank_np()
    assert abs(fb[512]).max() == 0.0
    fbank = np.zeros((128, 4, 64), np.float32)
    for ct in range(4):
        fbank[:, ct] = fb[ct * 128:(ct + 1) * 128]
    cmap["fbank"] = fbank.astype(BF16)

    w1f = fold(p["conv1"], p["g1"])
    cmap["w1m"] = conv1_mats(w1f)

    # biases: one fp32 [128, NB] array; column map
    cols = {}
    bias_cols = []
    b1 = np.asarray(p["b1"], np.float32)
    for mt in range(16):
        col = np.zeros(128, np.float32)
        for pp in range(128):
            col[pp] = b1[4 * mt + pp // 32]
        bias_cols.append(col)
    cols["conv1"] = 0

    def add_bias(name, bias):
        cols[name] = len(bias_cols)
        co = bias.shape[0]
        for cot in range((co + 127) // 128):
            col = np.zeros(128, np.float32)
            seg = bias[cot * 128:(cot + 1) * 128]
            col[:len(seg)] = seg
            bias_cols.append(col)

    cin = 64
    for li, (c, n) in enumerate(CFG):
        for bi in range(n):
            stride = 2 if (li > 0 and bi == 0) else 1
            bp = p["layers"][li][bi]
            pk1 = pack_conv_paired if f"w_{li}_{bi}_1" in PAIRED else pack_conv
            pk2 = pack_conv_paired if f"w_{li}_{bi}_2" in PAIRED else pack_conv
            cmap[f"w_{li}_{bi}_1"] = pk1(fold(bp["w1"], bp["g1"]))
            add_bias(f"w_{li}_{bi}_1", np.asarray(bp["b1"], np.float32))
            cmap[f"w_{li}_{bi}_2"] = pk2(fold(bp["w2"], bp["g2"]))
            add_bias(f"w_{li}_{bi}_2", np.asarray(bp["b2"], np.float32))
            if stride != 1 or cin != c:
                cmap[f"w_{li}_{bi}_d"] = pack_conv(fold(bp["wd"], bp["gd"]))
                add_bias(f"w_{li}_{bi}_d", np.asarray(bp["bd"], np.float32))
            cin = c
    # fc
    fcb = np.zeros(128, np.float32)
    fcb[:10] = np.asarray(p["fc_b"], np.float32)
    cols["fc"] = len(bias_cols)
    bias_cols.append(fcb)
    cmap["biases"] = np.stack(bias_cols, axis=1).astype(np.float32)  # [128, NB]

    fcw = np.zeros((128, 4, 10), np.float32)
    w = np.asarray(p["fc_w"], np.float32) / 26.0  # fold avgpool 1/(2*13)
    for ct in range(4):
        fcw[:, ct] = w[:, ct * 128:(ct + 1) * 128].T
    cmap["fcw"] = fcw

    # PCEN / input-bn scalars
    alpha = float(np.clip(np.asarray(p["alpha"]), 0.01, 0.99))
    delta = float(abs(np.asarray(p["delta"])) + EPS)
    r = float(np.clip(np.asarray(p["r"]), 0.01, 1.0))
    assert abs(r - 0.5) < 1e-6, "kernel assumes r=0.5 (sqrt)"
    s0 = float(np.asarray(p["inbn_g"])[0] / np.sqrt(1.0 + BN_EPS))
    c0 = float(np.asarray(p["inbn_b"])[0] - (delta ** r) * s0)
    meta = dict(alpha=alpha, delta=delta, s0=s0, c0=c0, cols=cols)
    return cmap, meta


def frames_for_shard(xs):
    """xs [B, 128000] fp32 -> [128, 8, B*T] bf16 (frames_T[p,c,(b,t)] = xp[b, 320t+128c+p])"""
    xp = np.pad(xs, ((0, 0), (512, 512)), mode="reflect")
    sw = np.lib.stride_tricks.sliding_window_view(xp, NFFT, axis=1)  # [B, L-1023, 1024]
    fr = sw[:, ::HOP][:, :T]                    # [B, T, 1024]
    fr = fr.transpose(2, 0, 1).reshape(8, 128, B * T)  # [c, p, bt]
    return np.ascontiguousarray(fr.transpose(1, 0, 2)).astype(BF16)


# ---------------------------------------------------------------- graph
def build_graph(cmap, meta, dbg=0):
    nc = bacc.Bacc("TRN2", target_bir_lowering=False, debug=False,
                   num_devices=NCORES)
    cols = meta["cols"]

    for v in (float(EPS), float(meta["delta"])):
        if (F32, v) not in nc.const_aps.aps:
            t_ = nc.alloc_sbuf_tensor(f"const-f32-{v}", [128, 1], F32)
            nc.gpsimd.memset(t_.ap(), v)
            nc.const_aps.aps[(F32, v)] = t_.ap()

    ext = {}
    ext["frames"] = nc.dram_tensor("frames", [128, 8, B * T], BF, kind="ExternalInput")
    for name, arr in cmap.items():
        dt = BF if arr.dtype == BF16 else F32
        ext[name] = nc.dram_tensor(name, list(arr.shape), dt, kind="ExternalInput")
    out_ext = nc.dram_tensor("out", [10, B], F32, kind="ExternalOutput")
    if dbg == 1:
        dbg_ext = nc.dram_tensor("dbg", [64, B, 407], BF, kind="ExternalOutput")
    elif dbg == 2:
        dbg_ext = nc.dram_tensor("dbg", [64, 1, B, 18, 103], BF, kind="ExternalOutput")
    elif dbg == 3:
        dbg_ext = nc.dram_tensor("dbg", [64, 1, B, 18, 103], BF, kind="ExternalOutput")

    with tile.TileContext(nc) as tc:
        import contextlib
        stack = contextlib.ExitStack()
        with stack:
            cpool = stack.enter_context(tc.tile_pool(name="consts", bufs=1))
            wpool = stack.enter_context(tc.tile_pool(name="w", bufs=4))
            pp = stack.enter_context(tc.tile_pool(name="psum", bufs=8, space="PSUM"))
            acts = stack.enter_context(tc.tile_pool(name="acts", bufs=1))

            biases_sb = cpool.tile([128, cmap["biases"].shape[1]], F32, tag="biases")
            nc.sync.dma_start(biases_sb[:], ext["biases"][:])
            fcw_sb = cpool.tile([128, 4, 10], F32, tag="fcw")
            nc.sync.dma_start(fcw_sb[:], ext["fcw"][:])
            fbank_sb = cpool.tile([128, 4, 64], BF, tag="fbank")
            nc.sync.dma_start(fbank_sb[:], ext["fbank"][:])

            z = cpool.tile([128, B, 407], BF, tag="z")         # conv1 input, W-padded, paired
            x_l1 = acts.tile([128, 1, B, 18, 103], BF, tag="xl1")  # post-maxpool, padded, paired

            def bias_ap(key, cot, pm):
                return biases_sb[0:pm, cols[key] + cot: cols[key] + cot + 1]

            # ---------------- frontend ----------------
            with tc.tile_pool(name="fe1", bufs=1) as fe1, \
                 tc.tile_pool(name="few", bufs=4) as few:
                warm = fe1.tile([1, 8], F32, tag="warm")
                warm_d = nc.dram_tensor("warm_sink", [1, 8], F32)
                nc.vector.memset(warm[:], 1.0)
                for wi, fn in enumerate((AF.Square, AF.Sqrt, AF.Ln, AF.Exp,
                                         AF.Relu, AF.Identity)):
                    nc.scalar.activation(warm[:, wi:wi + 1], warm[:, wi:wi + 1], fn)
                nc.sync.dma_start(warm_d[:], warm[:])  # keep warm-up live past DCE
                frames_sb = fe1.tile([128, 8, B * T], BF, tag="frames")
                for b in range(B):
                    nc.sync.dma_start(frames_sb[:, :, b * T:(b + 1) * T],
                                      ext["frames"][:, :, b * T:(b + 1) * T])
                dftm_sb = fe1.tile([128, 8, 2, NF], BF, tag="dftm")
                nc.sync.dma_start(dftm_sb[:], ext["dftm"][:])
                mag = fe1.tile([128, 4, B * T], BF, tag="mag")
                mel = fe1.tile([64, B * T], F32, tag="mel")
                A = fe1.tile([64, B * T], F32, tag="A")
                Bt = fe1.tile([64, B * T], F32, tag="Bt")
                scanc = fe1.tile([64, T], F32, tag="scanc")
                nc.vector.memset(scanc[:], 1.0 - S)

                # DFT + magnitude: sample-major so mel/PCEN pipeline per sample
                for b in range(B):
                    for mt in range(4):
                        ps_re = pp.tile([128, T], F32, tag="ps")
                        ps_im = pp.tile([128, T], F32, tag="ps")
                        for c in range(8):
                            rhs = frames_sb[:, c, b * T:(b + 1) * T]
                            nc.tensor.matmul(ps_re[:], dftm_sb[:, c, 0, mt * 128:(mt + 1) * 128],
                                             rhs, start=(c == 0), stop=(c == 7))
                            nc.tensor.matmul(ps_im[:], dftm_sb[:, c, 1, mt * 128:(mt + 1) * 128],
                                             rhs, start=(c == 0), stop=(c == 7))
                        sq1 = few.tile([128, T], F32, tag="sq")
                        sq2 = few.tile([128, T], F32, tag="sq")
                        nc.scalar.activation(sq1[:], ps_re[:], AF.Square)
                        nc.scalar.activation(sq2[:], ps_im[:], AF.Square)
                        nc.vector.tensor_add(sq1[:], sq1[:], sq2[:])
                        nc.scalar.activation(mag[:, mt, b * T:(b + 1) * T], sq1[:], AF.Sqrt)

                # mel projection: [64, T] per sample
                for b in range(B):
                    ps = pp.tile([64, T], F32, tag="ps")
                    for ct in range(4):
                        nc.tensor.matmul(ps[:], fbank_sb[:, ct, :],
                                         mag[:, ct, b * T:(b + 1) * T],
                                         start=(ct == 0), stop=(ct == 3))
                    nc.scalar.activation(mel[:, b * T:(b + 1) * T], ps[:], AF.Copy)

                # PCEN
                for b in range(B):
                    sl = slice(b * T, (b + 1) * T)
                    nc.vector.tensor_scalar_mul(A[:, sl], mel[:, sl], S)
                    nc.vector.tensor_copy(A[:, b * T:b * T + 1], mel[:, b * T:b * T + 1])
                    nc.vector.tensor_tensor_scan(
                        Bt[:, sl], scanc[:], A[:, sl],
                        mel[:, b * T:b * T + 1], ALU.mult, ALU.add)
                nc.vector.memset(z[:, :, 0:3], 0.0)
                nc.vector.memset(z[:, :, 404:407], 0.0)
                for b in range(B):
                    sl = slice(b * T, (b + 1) * T)
                    nc.scalar.activation(A[:, sl], Bt[:, sl], AF.Ln, bias=EPS)
                    nc.scalar.activation(Bt[:, sl], A[:, sl], AF.Exp, scale=-meta["alpha"])
                    nc.vector.tensor_mul(A[:, sl], mel[:, sl], Bt[:, sl])
                    nc.scalar.activation(Bt[:, sl], A[:, sl], AF.Sqrt, bias=meta["delta"])
                    nc.vector.tensor_scalar(
                        z[0:64, b, 3:404], Bt[:, sl],
                        meta["s0"], meta["c0"], ALU.mult, ALU.add)
                    # paired copy: rows 64:128 = rows 0:64 shifted left by 1
                    nc.sync.dma_start(z[64:128, b, 0:406], z[0:64, b, 1:407])

            if dbg == 1:
                nc.sync.dma_start(dbg_ext[:], z[:])

            # ---------------- conv1 + maxpool ----------------
            with tc.tile_pool(name="fe2", bufs=1) as fe2, \
                 tc.tile_pool(name="fe2w", bufs=16) as fe2w:
                w1_t = []
                for j in range(4):
                    t_ = fe2.tile([128, 2048], BF, tag=f"w1_{j}")
                    nc.scalar.dma_start(t_[:], ext["w1m"][j])
                    w1_t.append(t_)
                y1p = fe2.tile([64, B, 34, 203], BF, tag="y1p")
                nc.vector.memset(y1p[:, :, 0:1, :], 0.0)
                nc.vector.memset(y1p[:, :, 33:34, :], 0.0)
                nc.vector.memset(y1p[:, :, :, 0:1], 0.0)
                nc.vector.memset(y1p[:, :, :, 202:203], 0.0)
                tv = fe2.tile([64, B, 16, 203], BF, tag="tv")
                tw = fe2.tile([64, B, 16, 101], BF, tag="tw")
                nc.vector.memset(x_l1[:, 0, :, 0:1, :], 0.0)
                nc.vector.memset(x_l1[:, 0, :, 17:18, :], 0.0)
                nc.vector.memset(x_l1[:, 0, :, :, 0:1], 0.0)
                nc.vector.memset(x_l1[:, 0, :, :, 102:103], 0.0)
                for bp in (0, 2):
                    for mt in range(16):
                        ps = pp.tile([128, 2, 201], F32, tag="ps")
                        for j in range(4):
                            nc.tensor.matmul(ps[:], w1_t[j][:, mt * 128:(mt + 1) * 128],
                                             z[:, bp:bp + 2, 2 * j:2 * j + 401:2],
                                             start=(j == 0), stop=(j == 3))
                        t1 = fe2w.tile([128, 2, 201], BF, tag="c1t")
                        nc.scalar.activation(t1[:], ps[:], AF.Relu, bias=bias_ap("conv1", mt, 128))
                        nc.sync.dma_start(
                            y1p[4 * mt:4 * mt + 4, bp + 0, 1:33, 1:202], t1[:, 0, :])
                        nc.gpsimd.dma_start(
                            y1p[4 * mt:4 * mt + 4, bp + 1, 1:33, 1:202], t1[:, 1, :])
                    for b in (bp, bp + 1):
                        # maxpool 3x3 s2 p1 (inputs >= 0; zero pad is safe)
                        nc.vector.tensor_max(tv[:, b], y1p[:, b, 0:32:2, :], y1p[:, b, 1:33:2, :])
                        nc.vector.tensor_max(tv[:, b], tv[:, b], y1p[:, b, 2:34:2, :])
                        nc.vector.tensor_max(tw[:, b], tv[:, b, :, 0:202:2], tv[:, b, :, 1:203:2])
                        nc.vector.tensor_max(x_l1[0:64, 0, b, 1:17, 1:102], tw[:, b], tv[:, b, :, 2:203:2])
                        nc.sync.dma_start(x_l1[64:128, 0, b, :, 0:102], x_l1[0:64, 0, b, :, 1:103])

            if dbg == 2:
                nc.sync.dma_start(dbg_ext[:], x_l1[:])

            # ---------------- ResNet layers ----------------
            def do_conv(X, cin, cout, stride, ksz, Ho, Wo, nts, wname, writer,
                        paired=False):
                """X: padded input tile [P, CT, B, Hp, Wp] (pad=1 iff ksz==3).
                nts: list of ('all'|b, oh0, ohn). writer(psum, cot, nt).
                paired: cin=64, X has W-shifted copy on partitions 64:128;
                weight slots s=kh*2+j cover kw={2j, 2j+1}."""
                P = 128 if paired else min(cin, 128)
                CT = (cin + 127) // 128
                PM = min(cout, 128)
                COT = (cout + 127) // 128
                KK = 6 if paired else ksz * ksz
                off = 0 if ksz == 3 else 1  # 1x1 conv reads interior of padded input
                psums = {}
                for ct in range(CT):
                    wt = wpool.tile([P, KK, cout], BF, tag="w")
                    nc.scalar.dma_start(wt[:], ext[wname][ct])
                    for cot in range(COT):
                        for ni, nt in enumerate(nts):
                            bsel, oh0, ohn = nt
                            if ct == 0:
                                shape = [PM, B, ohn, Wo] if bsel == "all" else [PM, ohn, Wo]
                                psums[(cot, ni)] = pp.tile(shape, F32, tag="ps", name="cps")
                            ps = psums[(cot, ni)]
                            for kk in range(KK):
                                if paired:
                                    kh, j = divmod(kk, 2)
                                    w0 = 2 * j
                                else:
                                    kh, kw = divmod(kk, ksz)
                                    w0 = kw + off
                                h0 = kh + off + stride * oh0
                                hsl = slice(h0, h0 + stride * (ohn - 1) + 1, stride)
                                wsl = slice(w0, w0 + stride * (Wo - 1) + 1, stride)
                                if bsel == "all":
                                    rhs = X[0:P, ct, :, hsl, wsl]
                                else:
                                    rhs = X[0:P, ct, bsel, hsl, wsl]
                                nc.tensor.matmul(
                                    ps[:], wt[:, kk, cot * 128:cot * 128 + PM], rhs,
                                    start=(ct == 0 and kk == 0),
                                    stop=(ct == CT - 1 and kk == KK - 1))
                            if ct == CT - 1:
                                writer(ps, cot, nt)

            def interior(Xt, cot, nt, Ho, Wo, pad=1, pm=None):
                bsel, oh0, ohn = nt
                pm = Xt.shape[0] if pm is None else pm
                if bsel == "all":
                    return Xt[0:pm, cot, :, pad + oh0:pad + oh0 + ohn, pad:pad + Wo]
                return Xt[0:pm, cot, bsel, pad + oh0:pad + oh0 + ohn, pad:pad + Wo]

            def relu_writer(dest, key, Ho, Wo, pm):
                def w(ps, cot, nt):
                    nc.scalar.activation(interior(dest, cot, nt, Ho, Wo, pm=pm), ps[:],
                                         AF.Relu, bias=bias_ap(key, cot, pm))
                return w

            def ident_writer(dest, key, Ho, Wo, pm):
                def w(ps, cot, nt):
                    nc.scalar.activation(interior(dest, cot, nt, Ho, Wo, pad=0), ps[:],
                                         AF.Identity, bias=bias_ap(key, cot, pm))
                return w

            def res_writer(dest, key, scget, Ho, Wo, pm):
                def w(ps, cot, nt):
                    nc.vector.tensor_add(ps[:], ps[:], scget(cot, nt))
                    nc.scalar.activation(interior(dest, cot, nt, Ho, Wo, pm=pm), ps[:],
                                         AF.Relu, bias=bias_ap(key, cot, pm))
                return w

            def memset_border(Xt, P, CT, Hp, Wp):
                for ct in range(CT):
                    nc.vector.memset(Xt[0:P, ct, :, 0:1, :], 0.0)
                    nc.vector.memset(Xt[0:P, ct, :, Hp - 1:Hp, :], 0.0)
                    nc.vector.memset(Xt[0:P, ct, :, :, 0:1], 0.0)
                    nc.vector.memset(Xt[0:P, ct, :, :, Wp - 1:Wp], 0.0)

            x = x_l1
            cin = 64
            lpools = [stack.enter_context(tc.tile_pool(name=f"l{li}", bufs=1))
                      for li in range(4)]
            for li, (cout, nb) in enumerate(CFG):
                Ho, Wo = GEOM[li + 1]
                Hp, Wp = Ho + 2, Wo + 2
                P = min(cout, 128)
                COT = (cout + 127) // 128
                if li == 0:
                    nts = [(b, o, n) for b in range(B) for o, n in
                           [(0, 5), (5, 5), (10, 5), (15, 1)]]
                elif li == 1:
                    nts = [(b, 0, Ho) for b in range(B)]
                else:
                    nts = [("all", 0, Ho)]
                lp = lpools[li]
                PT = 128 if li == 0 else P  # layer-1 tiles carry the paired copy
                for bi in range(nb):
                    stride = 2 if (li > 0 and bi == 0) else 1
                    y = lp.tile([PT, COT, B, Hp, Wp], BF, tag=f"y{li}")
                    xo = lp.tile([PT, COT, B, Hp, Wp], BF, tag=f"x{li}_{bi % 2}")
                    if bi == 0:
                        memset_border(y, PT, COT, Hp, Wp)
                    if bi < 2:
                        memset_border(xo, PT, COT, Hp, Wp)

                    def cat(Xt, Hp=Hp, Wp=Wp):
                        for b in range(B):
                            nc.sync.dma_start(Xt[64:128, 0, b, :, 0:Wp - 1],
                                              Xt[0:64, 0, b, :, 1:Wp])

                    k1 = f"w_{li}_{bi}_1"
                    do_conv(x, cin, cout, stride, 3, Ho, Wo, nts, k1,
                            relu_writer(y, k1, Ho, Wo, P), paired=(k1 in PAIRED))
                    if li == 0:
                        cat(y)
                    if stride != 1 or cin != cout:
                        sc = lp.tile([P, COT, B, Ho, Wo], BF, tag=f"sc{li}")
                        kd = f"w_{li}_{bi}_d"
                        do_conv(x, cin, cout, stride, 1, Ho, Wo, nts, kd,
                                ident_writer(sc, kd, Ho, Wo, P))
                        scget = lambda cot, nt, sc=sc: interior(sc, cot, nt, Ho, Wo, pad=0)
                    else:
                        scget = lambda cot, nt, x=x: interior(x, cot, nt, Ho, Wo, pm=P)
                    k2 = f"w_{li}_{bi}_2"
                    do_conv(y, cout, cout, 1, 3, Ho, Wo, nts, k2,
                            res_writer(xo, k2, scget, Ho, Wo, P), paired=(k2 in PAIRED))
                    if li == 0:
                        cat(xo)
                    x = xo
                    cin = cout

            # ---------------- avgpool + fc ----------------
            feat = cpool.tile([128, 4, B], F32, tag="feat")
            for ct in range(4):
                nc.vector.tensor_reduce(feat[:, ct, :], x[:, ct, :, 1:3, 1:14],
                                        AX.XY, ALU.add)
            psf = pp.tile([10, B], F32, tag="ps")
            for ct in range(4):
                nc.tensor.matmul(psf[:], fcw_sb[:, ct, :], feat[:, ct, :],
                                 start=(ct == 0), stop=(ct == 3))
            out_sb = cpool.tile([10, B], F32, tag="outsb")
            nc.scalar.activation(out_sb[:], psf[:], AF.Identity,
                                 bias=bias_ap("fc", 0, 10))
            nc.sync.dma_start(out_ext[:], out_sb[:])

    nc.compile()
    return nc


# ---------------------------------------------------------------- entry
def kernel(x, params):
    x = np.asarray(x, np.float32)
    cmap, meta = host_prep(params)
    dbg = int(os.environ.get("AK_DBG", "0"))
    nc = build_graph(cmap, meta, dbg=dbg)
    in_maps = []
    for c in range(NCORES):
        m = {"frames": frames_for_shard(x[c * B:(c + 1) * B])}
        for k, v in cmap.items():
            m[k] = np.asarray(v)
        in_maps.append(m)
    res = run_bass_kernel_spmd(nc, in_maps, list(range(NCORES)),
                               trace=bool(int(os.environ.get("AK_TRACE", "0"))))
    outs = [res.results[c]["out"].T for c in range(NCORES)]  # [4, 10] each
    if dbg:
        kernel.dbg = [res.results[c].get("dbg") for c in range(NCORES)]
    kernel.exec_time_ns = res.exec_time_ns
    return np.concatenate(outs, axis=0).astype(np.float32)
